# revision 3
# baseline (speedup 1.0000x reference)
"""Trainium2 Bass kernel for nn_Classifier_42588895707508.

Computation (see reference):
    pool_k[b, h] = max_{s < eff_k[b]} x_k[b, s, h]      (k = 1, 2)
    out[b, c]    = sum_h pool_1[b,h] W[c,h] + pool_2[b,h] W[c, 768+h] + bias[c]
where eff_k[b] is derived from the mask m_k (index of first zero; 0 -> S).

Strategy (memory-bound, ragged sequences):
  * The valid region of each sample is a contiguous DRAM prefix. Only those
    bytes ever need to touch the device (~50% of the input on average).
  * Host computes eff from the masks, then packs, per core, the transposed
    valid blocks x_k[b, :eff, :].T (so h is on partitions, s on the free
    dim) into one dense array P[6, 128, R]: 6 chunks of 128 h-partitions,
    all rows' segments concatenated along the free dim.
  * Rows (= (kind, sample) pairs, 2*512 = 1024 of them) are distributed
    round-robin by length rank across the 8 cores, so every core gets an
    identical segment-width structure -> one SPMD program, perfectly
    balanced load.
  * Segments are grouped into equal-width groups (sorted by length, padded
    by at most a few columns) so one DVE reduce_max instruction with a
    [128, g, w] access pattern pools g rows at once.
  * The tiny linear layer runs on the tensor engine (fp32 matmul,
    K=128 per chunk accumulated in PSUM); per-row partial dot products
    are summed on the host (x1/x2 rows of one sample may land on
    different cores).
"""

import numpy as np

B, S, H, C = 512, 256, 768, 2
NCORES = 8
CH = H // 128  # 6 h-chunks of 128 partitions
KINDS = 2
SLOTS = B // NCORES  # 64 slots per kind per core
NEG = np.float32(-3.4028235e38)

# grouping cost model (element-column units): a padded column costs ~2
# (DMA + DVE), an extra reduce instruction costs ~58 DVE cycles
PAD_COST = 2.4
INSTR_COST = 140.0
TILE_W = 2048  # max free width of one SBUF data tile
RAMP = [512, 1024]  # smaller first tiles so DVE starts early
DATA_BUFS = 12


def _eff_lengths(m):
    am = np.argmin(np.asarray(m), axis=1)
    return np.where(am == 0, S, am).astype(np.int64)


def _plan_groups(widths):
    """Partition the (descending) width list into contiguous groups.

    Returns list of (start, n, gw) minimizing PAD_COST * padding +
    INSTR_COST per group, via O(n^2) DP.
    """
    n = len(widths)
    best = np.full(n + 1, np.inf)
    best[0] = 0.0
    prev = np.zeros(n + 1, dtype=np.int64)
    for i in range(1, n + 1):
        pad = 0.0
        for j in range(i - 1, -1, -1):
            # group covers slots j..i-1, width = widths[j] (max, sorted desc)
            pad = PAD_COST * (widths[j : i].size * widths[j] - widths[j : i].sum())
            cost = best[j] + pad + INSTR_COST
            if cost < best[i]:
                best[i] = cost
                prev[i] = j
    groups = []
    i = n
    while i > 0:
        j = prev[i]
        groups.append((j, i - j, int(widths[j])))
        i = j
    groups.reverse()
    return groups


def _build_program(groups_all, R, tiles):
    """Build the SPMD Bass program. groups_all: list of
    (col_off, out_col, n, w) covering all 128 output slots; tiles: list of
    (c0, c1, [groups...]) DMA tiles with group boundaries inside."""
    import concourse.bacc as bacc
    import concourse.mybir as mybir
    from concourse.tile import TileContext

    nc = bacc.Bacc("TRN2", target_bir_lowering=False, debug=False, num_devices=NCORES)
    p_in = nc.dram_tensor("p", [CH, 128, R], mybir.dt.float32, kind="ExternalInput")
    wt_in = nc.dram_tensor(
        "wt", [128, KINDS * CH, C], mybir.dt.float32, kind="ExternalInput"
    )
    out_d = nc.dram_tensor("out", [C, 2 * SLOTS], mybir.dt.float32, kind="ExternalOutput")

    with TileContext(nc) as tc:
        with (
            tc.tile_pool(name="data", bufs=DATA_BUFS) as data_pool,
            tc.tile_pool(name="pooled", bufs=1) as pooled_pool,
            tc.tile_pool(name="small", bufs=1) as small_pool,
            tc.tile_pool(name="psum", bufs=1, space="PSUM") as psum_pool,
        ):
            wt_t = small_pool.tile([128, KINDS * CH, C], mybir.dt.float32, tag="wt")
            nc.sync.dma_start(out=wt_t, in_=wt_in[:, :, :])

            pooled = [
                pooled_pool.tile([128, 2 * SLOTS], mybir.dt.float32, tag=f"pool{ch}", name=f"pool{ch}")
                for ch in range(CH)
            ]

            for ch in range(CH):
                for c0, c1, tgroups in tiles:
                    tw = c1 - c0
                    dt = data_pool.tile([128, TILE_W], mybir.dt.float32, tag="data")
                    nc.sync.dma_start(out=dt[:, :tw], in_=p_in[ch, :, c0:c1])
                    for col_off, out_col, gn, gw in tgroups:
                        a = col_off - c0
                        view = dt[:, a : a + gn * gw].rearrange(
                            "p (g w) -> p g w", w=gw
                        )
                        nc.vector.reduce_max(
                            out=pooled[ch][:, out_col : out_col + gn],
                            in_=view,
                            axis=mybir.AxisListType.X,
                        )

            out_sb = small_pool.tile([C, 2 * SLOTS], mybir.dt.float32, tag="osb")
            for k in range(KINDS):
                ps = psum_pool.tile([C, SLOTS], mybir.dt.float32, tag=f"ps{k}")
                for ch in range(CH):
                    nc.tensor.matmul(
                        ps,
                        lhsT=wt_t[:, k * CH + ch, :],
                        rhs=pooled[ch][:, k * SLOTS : (k + 1) * SLOTS],
                        start=(ch == 0),
                        stop=(ch == CH - 1),
                    )
                nc.vector.tensor_copy(
                    out=out_sb[:, k * SLOTS : (k + 1) * SLOTS], in_=ps
                )
            nc.sync.dma_start(out=out_d[:, :], in_=out_sb)

    nc.compile()
    return nc


def kernel(x1, x2, m1, m2, W, b, _run_opts=None):
    from concourse.bass_utils import run_bass_kernel_spmd

    x1 = np.asarray(x1)
    x2 = np.asarray(x2)
    W = np.asarray(W, dtype=np.float32)
    b = np.asarray(b, dtype=np.float32)
    xs = (x1, x2)
    effs = [_eff_lengths(m1), _eff_lengths(m2)]
    # descending length order per kind; slot i on core c holds sample
    # orders[k][i*NCORES + c]
    orders = [np.argsort(-effs[k], kind="stable") for k in range(KINDS)]
    # slot width = max eff among the 8 cores' samples of that slot
    slot_w = [
        effs[k][orders[k][:: NCORES]].astype(np.int64) for k in range(KINDS)
    ]  # [64] each, descending

    # group slots (per kind) into equal-width reduce groups
    col = 0
    groups_all = []  # (col_off, out_col, n_slots, width) in slot order
    slot_cols = [np.zeros(SLOTS, dtype=np.int64) for _ in range(KINDS)]
    slot_gw = [np.zeros(SLOTS, dtype=np.int64) for _ in range(KINDS)]
    for k in range(KINDS):
        for start, n, gw in _plan_groups(slot_w[k]):
            groups_all.append((col, k * SLOTS + start, n, gw))
            for j in range(n):
                slot_cols[k][start + j] = col + j * gw
                slot_gw[k][start + j] = gw
            col += n * gw
    R = col

    # DMA tiles: whole groups per tile; the first few tiles are smaller
    # (RAMP) so the vector engine starts reducing as early as possible
    tiles = []
    cur = None
    for g in groups_all:
        col_off, _, gn, gw = g
        gwidth = gn * gw
        cap = RAMP[len(tiles)] if len(tiles) < len(RAMP) else TILE_W
        if cur is not None and (col_off + gwidth - cur[0]) <= cap:
            cur[1] = col_off + gwidth
            cur[2].append(g)
        else:
            if cur is not None:
                tiles.append(tuple(cur))
            cur = [col_off, col_off + gwidth, [g]]
    if cur is not None:
        tiles.append(tuple(cur))
    assert all(c1 - c0 <= TILE_W for c0, c1, _ in tiles)
    print(f"kernel plan: R={R} bytes/core={R*CH*128*4/1e6:.1f}MB "
          f"groups={len(groups_all)} tiles={len(tiles)}")

    # pack per-core data: P[core][ch, p, col] = x_k[b, s, ch*128+p]
    packs = np.full((NCORES, CH, 128, R), NEG, dtype=np.float32)
    for k in range(KINDS):
        xk, eff, order = xs[k], effs[k], orders[k]
        for i in range(SLOTS):
            off = slot_cols[k][i]
            for c in range(NCORES):
                bidx = order[i * NCORES + c]
                e = int(eff[bidx])
                dst = packs[c].reshape(CH * 128, R)
                dst[:, off : off + e] = xk[bidx, :e, :].T

    # weights, laid out so lhsT slices are [128 (h), C] per (kind, chunk)
    wtp = np.ascontiguousarray(
        W.reshape(C, KINDS, CH, 128).transpose(3, 1, 2, 0)
    ).reshape(128, KINDS * CH, C)

    nc = _build_program(groups_all, R, tiles)
    in_maps = [{"p": packs[c], "wt": wtp} for c in range(NCORES)]
    res = run_bass_kernel_spmd(
        nc, in_maps, core_ids=list(range(NCORES)), **(_run_opts or {})
    )

    # combine per-row partials
    out_full = np.zeros((B, C), dtype=np.float32)
    res_all = np.stack([res.results[c]["out"] for c in range(NCORES)])  # [8, C, 128]
    for k in range(KINDS):
        part = res_all[:, :, k * SLOTS : (k + 1) * SLOTS]  # [core, C, slot]
        part = part.transpose(2, 0, 1).reshape(B, C)  # [(slot, core), C]
        out_full[orders[k]] += part
    out_full += b[None, :]
    if _run_opts is not None:
        kernel._last_res = res
    return out_full


# revision 4
# speedup vs baseline: 1.0112x; 1.0112x over previous
"""Trainium2 Bass kernel for nn_Classifier_42588895707508.

Computation (see reference):
    pool_k[b, h] = max_{s < eff_k[b]} x_k[b, s, h]      (k = 1, 2)
    out[b, c]    = sum_h pool_1[b,h] W[c,h] + pool_2[b,h] W[c, 768+h] + bias[c]
where eff_k[b] is derived from the mask m_k (index of first zero; 0 -> S).

Strategy (memory-bound, ragged sequences):
  * The valid region of each sample is a contiguous DRAM prefix. Only those
    bytes ever need to touch the device (~50% of the input on average).
  * Host computes eff from the masks, then packs, per core, the transposed
    valid blocks x_k[b, :eff, :].T (so h is on partitions, s on the free
    dim) into one dense array P[6, 128, R]: 6 chunks of 128 h-partitions,
    all rows' segments concatenated along the free dim.
  * Rows (= (kind, sample) pairs, 2*512 = 1024 of them) are distributed
    round-robin by length rank across the 8 cores, so every core gets an
    identical segment-width structure -> one SPMD program, perfectly
    balanced load.
  * Segments are grouped into equal-width groups (sorted by length, padded
    by at most a few columns) so one DVE reduce_max instruction with a
    [128, g, w] access pattern pools g rows at once.
  * The tiny linear layer runs on the tensor engine (fp32 matmul,
    K=128 per chunk accumulated in PSUM); per-row partial dot products
    are summed on the host (x1/x2 rows of one sample may land on
    different cores).
"""

import numpy as np

B, S, H, C = 512, 256, 768, 2
NCORES = 8
CH = H // 128  # 6 h-chunks of 128 partitions
KINDS = 2
SLOTS = B // NCORES  # 64 slots per kind per core
NEG = np.float32(-3.4028235e38)

# grouping cost model (element-column units): a padded column costs ~2
# (DMA + DVE), an extra reduce instruction costs ~58 DVE cycles
PAD_COST = 1.0
INSTR_COST = 40.0
TILE_W = 2048  # max free width of one SBUF data tile
RAMP = [512, 1024]  # smaller first tiles so DVE starts early
DATA_BUFS = 12


def _eff_lengths(m):
    am = np.argmin(np.asarray(m), axis=1)
    return np.where(am == 0, S, am).astype(np.int64)


def _plan_groups(widths):
    """Partition the (descending) width list into contiguous groups.

    Returns list of (start, n, gw) minimizing PAD_COST * padding +
    INSTR_COST per group, via O(n^2) DP.
    """
    n = len(widths)
    best = np.full(n + 1, np.inf)
    best[0] = 0.0
    prev = np.zeros(n + 1, dtype=np.int64)
    for i in range(1, n + 1):
        pad = 0.0
        for j in range(i - 1, -1, -1):
            # group covers slots j..i-1, width = widths[j] (max, sorted desc)
            pad = PAD_COST * (widths[j : i].size * widths[j] - widths[j : i].sum())
            cost = best[j] + pad + INSTR_COST
            if cost < best[i]:
                best[i] = cost
                prev[i] = j
    groups = []
    i = n
    while i > 0:
        j = prev[i]
        groups.append((j, i - j, int(widths[j])))
        i = j
    groups.reverse()
    return groups


def _build_program(groups_all, R, tiles):
    """Build the SPMD Bass program. groups_all: list of
    (col_off, out_col, n, w) covering all 128 output slots; tiles: list of
    (c0, c1, [groups...]) DMA tiles with group boundaries inside."""
    import concourse.bacc as bacc
    import concourse.mybir as mybir
    from concourse.tile import TileContext

    nc = bacc.Bacc("TRN2", target_bir_lowering=False, debug=False, num_devices=NCORES)
    p_in = nc.dram_tensor("p", [CH, 128, R], mybir.dt.float32, kind="ExternalInput")
    wt_in = nc.dram_tensor(
        "wt", [128, KINDS * CH, C], mybir.dt.float32, kind="ExternalInput"
    )
    out_d = nc.dram_tensor("out", [C, 2 * SLOTS], mybir.dt.float32, kind="ExternalOutput")

    with TileContext(nc) as tc:
        with (
            tc.tile_pool(name="data", bufs=DATA_BUFS) as data_pool,
            tc.tile_pool(name="pooled", bufs=1) as pooled_pool,
            tc.tile_pool(name="small", bufs=1) as small_pool,
            tc.tile_pool(name="psum", bufs=1, space="PSUM") as psum_pool,
        ):
            wt_t = small_pool.tile([128, KINDS * CH, C], mybir.dt.float32, tag="wt")
            nc.sync.dma_start(out=wt_t, in_=wt_in[:, :, :])

            pooled = [
                pooled_pool.tile([128, 2 * SLOTS], mybir.dt.float32, tag=f"pool{ch}", name=f"pool{ch}")
                for ch in range(CH)
            ]

            for ch in range(CH):
                for c0, c1, tgroups in tiles:
                    tw = c1 - c0
                    dt = data_pool.tile([128, TILE_W], mybir.dt.float32, tag="data")
                    nc.sync.dma_start(out=dt[:, :tw], in_=p_in[ch, :, c0:c1])
                    for col_off, out_col, gn, gw in tgroups:
                        a = col_off - c0
                        view = dt[:, a : a + gn * gw].rearrange(
                            "p (g w) -> p g w", w=gw
                        )
                        nc.vector.reduce_max(
                            out=pooled[ch][:, out_col : out_col + gn],
                            in_=view,
                            axis=mybir.AxisListType.X,
                        )

            out_sb = small_pool.tile([C, 2 * SLOTS], mybir.dt.float32, tag="osb")
            for k in range(KINDS):
                ps = psum_pool.tile([C, SLOTS], mybir.dt.float32, tag=f"ps{k}")
                for ch in range(CH):
                    nc.tensor.matmul(
                        ps,
                        lhsT=wt_t[:, k * CH + ch, :],
                        rhs=pooled[ch][:, k * SLOTS : (k + 1) * SLOTS],
                        start=(ch == 0),
                        stop=(ch == CH - 1),
                    )
                nc.vector.tensor_copy(
                    out=out_sb[:, k * SLOTS : (k + 1) * SLOTS], in_=ps
                )
            nc.sync.dma_start(out=out_d[:, :], in_=out_sb)

    nc.compile()
    return nc


def kernel(x1, x2, m1, m2, W, b, _run_opts=None):
    from concourse.bass_utils import run_bass_kernel_spmd

    x1 = np.asarray(x1)
    x2 = np.asarray(x2)
    W = np.asarray(W, dtype=np.float32)
    b = np.asarray(b, dtype=np.float32)
    xs = (x1, x2)
    effs = [_eff_lengths(m1), _eff_lengths(m2)]
    # descending length order per kind; slot i on core c holds sample
    # orders[k][i*NCORES + c]
    orders = [np.argsort(-effs[k], kind="stable") for k in range(KINDS)]
    # slot width = max eff among the 8 cores' samples of that slot
    slot_w = [
        effs[k][orders[k][:: NCORES]].astype(np.int64) for k in range(KINDS)
    ]  # [64] each, descending

    # group slots (per kind) into equal-width reduce groups
    col = 0
    groups_all = []  # (col_off, out_col, n_slots, width) in slot order
    slot_cols = [np.zeros(SLOTS, dtype=np.int64) for _ in range(KINDS)]
    slot_gw = [np.zeros(SLOTS, dtype=np.int64) for _ in range(KINDS)]
    for k in range(KINDS):
        for start, n, gw in _plan_groups(slot_w[k]):
            groups_all.append((col, k * SLOTS + start, n, gw))
            for j in range(n):
                slot_cols[k][start + j] = col + j * gw
                slot_gw[k][start + j] = gw
            col += n * gw
    R = col

    # DMA tiles: whole groups per tile; the first few tiles are smaller
    # (RAMP) so the vector engine starts reducing as early as possible
    tiles = []
    cur = None
    for g in groups_all:
        col_off, _, gn, gw = g
        gwidth = gn * gw
        cap = RAMP[len(tiles)] if len(tiles) < len(RAMP) else TILE_W
        if cur is not None and (col_off + gwidth - cur[0]) <= cap:
            cur[1] = col_off + gwidth
            cur[2].append(g)
        else:
            if cur is not None:
                tiles.append(tuple(cur))
            cur = [col_off, col_off + gwidth, [g]]
    if cur is not None:
        tiles.append(tuple(cur))
    assert all(c1 - c0 <= TILE_W for c0, c1, _ in tiles)
    print(f"kernel plan: R={R} bytes/core={R*CH*128*4/1e6:.1f}MB "
          f"groups={len(groups_all)} tiles={len(tiles)}")

    # pack per-core data: P[core][ch, p, col] = x_k[b, s, ch*128+p]
    packs = np.full((NCORES, CH, 128, R), NEG, dtype=np.float32)
    for k in range(KINDS):
        xk, eff, order = xs[k], effs[k], orders[k]
        for i in range(SLOTS):
            off = slot_cols[k][i]
            for c in range(NCORES):
                bidx = order[i * NCORES + c]
                e = int(eff[bidx])
                dst = packs[c].reshape(CH * 128, R)
                dst[:, off : off + e] = xk[bidx, :e, :].T

    # weights, laid out so lhsT slices are [128 (h), C] per (kind, chunk)
    wtp = np.ascontiguousarray(
        W.reshape(C, KINDS, CH, 128).transpose(3, 1, 2, 0)
    ).reshape(128, KINDS * CH, C)

    nc = _build_program(groups_all, R, tiles)
    in_maps = [{"p": packs[c], "wt": wtp} for c in range(NCORES)]
    res = run_bass_kernel_spmd(
        nc, in_maps, core_ids=list(range(NCORES)), **(_run_opts or {})
    )

    # combine per-row partials
    out_full = np.zeros((B, C), dtype=np.float32)
    res_all = np.stack([res.results[c]["out"] for c in range(NCORES)])  # [8, C, 128]
    for k in range(KINDS):
        part = res_all[:, :, k * SLOTS : (k + 1) * SLOTS]  # [core, C, slot]
        part = part.transpose(2, 0, 1).reshape(B, C)  # [(slot, core), C]
        out_full[orders[k]] += part
    out_full += b[None, :]
    if _run_opts is not None:
        kernel._last_res = res
    return out_full


# revision 5
# speedup vs baseline: 1.0149x; 1.0037x over previous
"""Trainium2 Bass kernel for nn_Classifier_42588895707508.

Computation (see reference):
    pool_k[b, h] = max_{s < eff_k[b]} x_k[b, s, h]      (k = 1, 2)
    out[b, c]    = sum_h pool_1[b,h] W[c,h] + pool_2[b,h] W[c, 768+h] + bias[c]
where eff_k[b] is derived from the mask m_k (index of first zero; 0 -> S).

Strategy (memory-bound, ragged sequences):
  * The valid region of each sample is a contiguous DRAM prefix. Only those
    bytes ever need to touch the device (~50% of the input on average).
  * Host computes eff from the masks, then packs, per core, the transposed
    valid blocks x_k[b, :eff, :].T (so h is on partitions, s on the free
    dim) into one dense array P[6, 128, R]: 6 chunks of 128 h-partitions,
    all rows' segments concatenated along the free dim.
  * Rows (= (kind, sample) pairs, 2*512 = 1024 of them) are distributed
    round-robin by length rank across the 8 cores, so every core gets an
    identical segment-width structure -> one SPMD program, perfectly
    balanced load.
  * Segments are grouped into equal-width groups (sorted by length, padded
    by at most a few columns) so one DVE reduce_max instruction with a
    [128, g, w] access pattern pools g rows at once.
  * The tiny linear layer runs on the tensor engine (fp32 matmul,
    K=128 per chunk accumulated in PSUM); per-row partial dot products
    are summed on the host (x1/x2 rows of one sample may land on
    different cores).
"""

import numpy as np

B, S, H, C = 512, 256, 768, 2
NCORES = 8
CH = H // 128  # 6 h-chunks of 128 partitions
KINDS = 2
SLOTS = B // NCORES  # 64 slots per kind per core
NEG = np.float32(-3.4028235e38)

# grouping cost model (element-column units): a padded column costs ~2
# (DMA + DVE), an extra reduce instruction costs ~58 DVE cycles
PAD_COST = 1.0
INSTR_COST = 40.0
TILE_W = 2048  # max free width of one SBUF data tile
RAMP = [512, 1024]  # smaller first tiles so DVE starts early
DATA_BUFS = 12


def _eff_lengths(m):
    am = np.argmin(np.asarray(m), axis=1)
    return np.where(am == 0, S, am).astype(np.int64)


def _plan_groups(widths):
    """Partition the (descending) width list into contiguous groups.

    Returns list of (start, n, gw) minimizing PAD_COST * padding +
    INSTR_COST per group, via O(n^2) DP.
    """
    n = len(widths)
    best = np.full(n + 1, np.inf)
    best[0] = 0.0
    prev = np.zeros(n + 1, dtype=np.int64)
    for i in range(1, n + 1):
        pad = 0.0
        for j in range(i - 1, -1, -1):
            # group covers slots j..i-1, width = widths[j] (max, sorted desc)
            pad = PAD_COST * (widths[j : i].size * widths[j] - widths[j : i].sum())
            cost = best[j] + pad + INSTR_COST
            if cost < best[i]:
                best[i] = cost
                prev[i] = j
    groups = []
    i = n
    while i > 0:
        j = prev[i]
        groups.append((j, i - j, int(widths[j])))
        i = j
    groups.reverse()
    return groups


def _build_program(groups_all, R, tiles):
    """Build the SPMD Bass program. groups_all: list of
    (col_off, out_col, n, w) covering all 128 output slots; tiles: list of
    (c0, c1, [groups...]) DMA tiles with group boundaries inside."""
    import concourse.bacc as bacc
    import concourse.mybir as mybir
    from concourse.tile import TileContext

    nc = bacc.Bacc("TRN2", target_bir_lowering=False, debug=False, num_devices=NCORES)
    p_in = nc.dram_tensor("p", [CH, 128, R], mybir.dt.float32, kind="ExternalInput")
    wt_in = nc.dram_tensor(
        "wt", [128, KINDS * CH, C], mybir.dt.float32, kind="ExternalInput"
    )
    out_d = nc.dram_tensor("out", [C, 2 * SLOTS], mybir.dt.float32, kind="ExternalOutput")

    with TileContext(nc) as tc:
        with (
            tc.tile_pool(name="data", bufs=DATA_BUFS) as data_pool,
            tc.tile_pool(name="pooled", bufs=1) as pooled_pool,
            tc.tile_pool(name="small", bufs=1) as small_pool,
            tc.tile_pool(name="psum", bufs=1, space="PSUM") as psum_pool,
        ):
            wt_t = small_pool.tile([128, KINDS * CH, C], mybir.dt.float32, tag="wt")
            nc.sync.dma_start(out=wt_t, in_=wt_in[:, :, :])

            pooled = [
                pooled_pool.tile([128, 2 * SLOTS], mybir.dt.float32, tag=f"pool{ch}", name=f"pool{ch}")
                for ch in range(CH)
            ]

            for ch in range(CH):
                for c0, c1, tgroups in tiles:
                    tw = c1 - c0
                    dt = data_pool.tile([128, TILE_W], mybir.dt.float32, tag="data")
                    nc.sync.dma_start(out=dt[:, :tw], in_=p_in[ch, :, c0:c1])
                    for col_off, out_col, gn, gw in tgroups:
                        a = col_off - c0
                        view = dt[:, a : a + gn * gw].rearrange(
                            "p (g w) -> p g w", w=gw
                        )
                        nc.vector.reduce_max(
                            out=pooled[ch][:, out_col : out_col + gn],
                            in_=view,
                            axis=mybir.AxisListType.X,
                        )

            out_sb = small_pool.tile([C, 2 * SLOTS], mybir.dt.float32, tag="osb")
            for k in range(KINDS):
                ps = psum_pool.tile([C, SLOTS], mybir.dt.float32, tag=f"ps{k}")
                for ch in range(CH):
                    nc.tensor.matmul(
                        ps,
                        lhsT=wt_t[:, k * CH + ch, :],
                        rhs=pooled[ch][:, k * SLOTS : (k + 1) * SLOTS],
                        start=(ch == 0),
                        stop=(ch == CH - 1),
                    )
                nc.vector.tensor_copy(
                    out=out_sb[:, k * SLOTS : (k + 1) * SLOTS], in_=ps
                )
            nc.sync.dma_start(out=out_d[:, :], in_=out_sb)

    nc.compile()
    return nc


_NC_CACHE = {}


def kernel(x1, x2, m1, m2, W, b, _run_opts=None):
    from concourse.bass_utils import run_bass_kernel_spmd

    x1 = np.asarray(x1)
    x2 = np.asarray(x2)
    W = np.asarray(W, dtype=np.float32)
    b = np.asarray(b, dtype=np.float32)
    xs = (x1, x2)
    effs = [_eff_lengths(m1), _eff_lengths(m2)]
    # descending length order per kind; slot i on core c holds sample
    # orders[k][i*NCORES + c]
    orders = [np.argsort(-effs[k], kind="stable") for k in range(KINDS)]
    # slot width = max eff among the 8 cores' samples of that slot
    slot_w = [
        effs[k][orders[k][:: NCORES]].astype(np.int64) for k in range(KINDS)
    ]  # [64] each, descending

    # group slots (per kind) into equal-width reduce groups
    col = 0
    groups_all = []  # (col_off, out_col, n_slots, width) in slot order
    slot_cols = [np.zeros(SLOTS, dtype=np.int64) for _ in range(KINDS)]
    slot_gw = [np.zeros(SLOTS, dtype=np.int64) for _ in range(KINDS)]
    for k in range(KINDS):
        for start, n, gw in _plan_groups(slot_w[k]):
            groups_all.append((col, k * SLOTS + start, n, gw))
            for j in range(n):
                slot_cols[k][start + j] = col + j * gw
                slot_gw[k][start + j] = gw
            col += n * gw
    R = col

    # DMA tiles: whole groups per tile; the first few tiles are smaller
    # (RAMP) so the vector engine starts reducing as early as possible
    tiles = []
    cur = None
    for g in groups_all:
        col_off, _, gn, gw = g
        gwidth = gn * gw
        cap = RAMP[len(tiles)] if len(tiles) < len(RAMP) else TILE_W
        if cur is not None and (col_off + gwidth - cur[0]) <= cap:
            cur[1] = col_off + gwidth
            cur[2].append(g)
        else:
            if cur is not None:
                tiles.append(tuple(cur))
            cur = [col_off, col_off + gwidth, [g]]
    if cur is not None:
        tiles.append(tuple(cur))
    assert all(c1 - c0 <= TILE_W for c0, c1, _ in tiles)

    # pack per-core data: P[core][ch, p, col] = x_k[b, s, ch*128+p]
    packs = np.full((NCORES, CH, 128, R), NEG, dtype=np.float32)
    for k in range(KINDS):
        xk, eff, order = xs[k], effs[k], orders[k]
        for i in range(SLOTS):
            off = slot_cols[k][i]
            for c in range(NCORES):
                bidx = order[i * NCORES + c]
                e = int(eff[bidx])
                dst = packs[c].reshape(CH * 128, R)
                dst[:, off : off + e] = xk[bidx, :e, :].T

    # weights, laid out so lhsT slices are [128 (h), C] per (kind, chunk)
    wtp = np.ascontiguousarray(
        W.reshape(C, KINDS, CH, 128).transpose(3, 1, 2, 0)
    ).reshape(128, KINDS * CH, C)

    key = (R, tuple(groups_all), tuple((c0, c1) for c0, c1, _ in tiles))
    nc = _NC_CACHE.get(key)
    if nc is None:
        nc = _build_program(groups_all, R, tiles)
        _NC_CACHE[key] = nc
    in_maps = [{"p": packs[c], "wt": wtp} for c in range(NCORES)]
    res = run_bass_kernel_spmd(
        nc, in_maps, core_ids=list(range(NCORES)), **(_run_opts or {})
    )

    # combine per-row partials
    out_full = np.zeros((B, C), dtype=np.float32)
    res_all = np.stack([res.results[c]["out"] for c in range(NCORES)])  # [8, C, 128]
    for k in range(KINDS):
        part = res_all[:, :, k * SLOTS : (k + 1) * SLOTS]  # [core, C, slot]
        part = part.transpose(2, 0, 1).reshape(B, C)  # [(slot, core), C]
        out_full[orders[k]] += part
    out_full += b[None, :]
    if _run_opts is not None:
        kernel._last_res = res
    return out_full


# revision 6
# speedup vs baseline: 1.0226x; 1.0076x over previous
"""Trainium2 Bass kernel for nn_Classifier_42588895707508.

Computation (see reference):
    pool_k[b, h] = max_{s < eff_k[b]} x_k[b, s, h]      (k = 1, 2)
    out[b, c]    = sum_h pool_1[b,h] W[c,h] + pool_2[b,h] W[c, 768+h] + bias[c]
where eff_k[b] is derived from the mask m_k (index of first zero; 0 -> S).

Strategy (memory-bound, ragged sequences):
  * The valid region of each sample is a contiguous DRAM prefix. Only those
    bytes ever need to touch the device (~50% of the input on average).
  * Host computes eff from the masks, then packs, per core, the transposed
    valid blocks x_k[b, :eff, :].T (so h is on partitions, s on the free
    dim) into one dense array P[6, 128, R]: 6 chunks of 128 h-partitions,
    all rows' segments concatenated along the free dim.
  * Rows (= (kind, sample) pairs, 2*512 = 1024 of them) are distributed
    round-robin by length rank across the 8 cores, so every core gets an
    identical segment-width structure -> one SPMD program, perfectly
    balanced load.
  * Segments are grouped into equal-width groups (sorted by length, padded
    by at most a few columns) so one DVE reduce_max instruction with a
    [128, g, w] access pattern pools g rows at once.
  * The tiny linear layer runs on the tensor engine (fp32 matmul,
    K=128 per chunk accumulated in PSUM); per-row partial dot products
    are summed on the host (x1/x2 rows of one sample may land on
    different cores).
"""

import numpy as np

B, S, H, C = 512, 256, 768, 2
NCORES = 8
CH = H // 128  # 6 h-chunks of 128 partitions
KINDS = 2
SLOTS = B // NCORES  # 64 slots per kind per core
NEG = np.float32(-3.4028235e38)

# grouping cost model (element-column units): a padded column costs ~2
# (DMA + DVE), an extra reduce instruction costs ~58 DVE cycles
PAD_COST = 1.0
INSTR_COST = 30.0
TILE_W = 2048  # max free width of one SBUF data tile
RAMP = [512, 1024]  # smaller first tiles so DVE starts early
DATA_BUFS = 12


def _eff_lengths(m):
    am = np.argmin(np.asarray(m), axis=1)
    return np.where(am == 0, S, am).astype(np.int64)


def _plan_groups(widths):
    """Partition the (descending) width list into contiguous groups.

    Returns list of (start, n, gw) minimizing PAD_COST * padding +
    INSTR_COST per group, via O(n^2) DP.
    """
    n = len(widths)
    best = np.full(n + 1, np.inf)
    best[0] = 0.0
    prev = np.zeros(n + 1, dtype=np.int64)
    for i in range(1, n + 1):
        pad = 0.0
        for j in range(i - 1, -1, -1):
            # group covers slots j..i-1, width = widths[j] (max, sorted desc)
            pad = PAD_COST * (widths[j : i].size * widths[j] - widths[j : i].sum())
            cost = best[j] + pad + INSTR_COST
            if cost < best[i]:
                best[i] = cost
                prev[i] = j
    groups = []
    i = n
    while i > 0:
        j = prev[i]
        groups.append((j, i - j, int(widths[j])))
        i = j
    groups.reverse()
    return groups


def _build_program(groups_all, R, tiles):
    """Build the SPMD Bass program. groups_all: list of
    (col_off, out_col, n, w) covering all 128 output slots; tiles: list of
    (c0, c1, [groups...]) DMA tiles with group boundaries inside."""
    import concourse.bacc as bacc
    import concourse.mybir as mybir
    from concourse.tile import TileContext

    nc = bacc.Bacc("TRN2", target_bir_lowering=False, debug=False, num_devices=NCORES)
    p_in = nc.dram_tensor("p", [CH, 128, R], mybir.dt.float32, kind="ExternalInput")
    wt_in = nc.dram_tensor(
        "wt", [128, KINDS * CH, C], mybir.dt.float32, kind="ExternalInput"
    )
    out_d = nc.dram_tensor("out", [C, 2 * SLOTS], mybir.dt.float32, kind="ExternalOutput")

    with TileContext(nc) as tc:
        with (
            tc.tile_pool(name="data", bufs=DATA_BUFS) as data_pool,
            tc.tile_pool(name="pooled", bufs=1) as pooled_pool,
            tc.tile_pool(name="small", bufs=1) as small_pool,
            tc.tile_pool(name="psum", bufs=1, space="PSUM") as psum_pool,
        ):
            wt_t = small_pool.tile([128, KINDS * CH, C], mybir.dt.float32, tag="wt")
            nc.sync.dma_start(out=wt_t, in_=wt_in[:, :, :])

            pooled = [
                pooled_pool.tile([128, 2 * SLOTS], mybir.dt.float32, tag=f"pool{ch}", name=f"pool{ch}")
                for ch in range(CH)
            ]

            for ch in range(CH):
                for c0, c1, tgroups in tiles:
                    tw = c1 - c0
                    dt = data_pool.tile([128, TILE_W], mybir.dt.float32, tag="data")
                    nc.sync.dma_start(out=dt[:, :tw], in_=p_in[ch, :, c0:c1])
                    for col_off, out_col, gn, gw in tgroups:
                        a = col_off - c0
                        view = dt[:, a : a + gn * gw].rearrange(
                            "p (g w) -> p g w", w=gw
                        )
                        nc.vector.reduce_max(
                            out=pooled[ch][:, out_col : out_col + gn],
                            in_=view,
                            axis=mybir.AxisListType.X,
                        )

            out_sb = small_pool.tile([C, 2 * SLOTS], mybir.dt.float32, tag="osb")
            for k in range(KINDS):
                ps = psum_pool.tile([C, SLOTS], mybir.dt.float32, tag=f"ps{k}")
                for ch in range(CH):
                    nc.tensor.matmul(
                        ps,
                        lhsT=wt_t[:, k * CH + ch, :],
                        rhs=pooled[ch][:, k * SLOTS : (k + 1) * SLOTS],
                        start=(ch == 0),
                        stop=(ch == CH - 1),
                    )
                nc.vector.tensor_copy(
                    out=out_sb[:, k * SLOTS : (k + 1) * SLOTS], in_=ps
                )
            nc.sync.dma_start(out=out_d[:, :], in_=out_sb)

    nc.compile()
    return nc


_NC_CACHE = {}


def kernel(x1, x2, m1, m2, W, b, _run_opts=None):
    from concourse.bass_utils import run_bass_kernel_spmd

    x1 = np.asarray(x1)
    x2 = np.asarray(x2)
    W = np.asarray(W, dtype=np.float32)
    b = np.asarray(b, dtype=np.float32)
    xs = (x1, x2)
    effs = [_eff_lengths(m1), _eff_lengths(m2)]
    # descending length order per kind; slot i on core c holds sample
    # orders[k][i*NCORES + c]
    orders = [np.argsort(-effs[k], kind="stable") for k in range(KINDS)]
    # slot width = max eff among the 8 cores' samples of that slot
    slot_w = [
        effs[k][orders[k][:: NCORES]].astype(np.int64) for k in range(KINDS)
    ]  # [64] each, descending

    # group slots (per kind) into equal-width reduce groups
    col = 0
    groups_all = []  # (col_off, out_col, n_slots, width) in slot order
    slot_cols = [np.zeros(SLOTS, dtype=np.int64) for _ in range(KINDS)]
    slot_gw = [np.zeros(SLOTS, dtype=np.int64) for _ in range(KINDS)]
    for k in range(KINDS):
        for start, n, gw in _plan_groups(slot_w[k]):
            groups_all.append((col, k * SLOTS + start, n, gw))
            for j in range(n):
                slot_cols[k][start + j] = col + j * gw
                slot_gw[k][start + j] = gw
            col += n * gw
    R = col

    # DMA tiles: whole groups per tile; the first few tiles are smaller
    # (RAMP) so the vector engine starts reducing as early as possible
    tiles = []
    cur = None
    for g in groups_all:
        col_off, _, gn, gw = g
        gwidth = gn * gw
        cap = RAMP[len(tiles)] if len(tiles) < len(RAMP) else TILE_W
        if cur is not None and (col_off + gwidth - cur[0]) <= cap:
            cur[1] = col_off + gwidth
            cur[2].append(g)
        else:
            if cur is not None:
                tiles.append(tuple(cur))
            cur = [col_off, col_off + gwidth, [g]]
    if cur is not None:
        tiles.append(tuple(cur))
    assert all(c1 - c0 <= TILE_W for c0, c1, _ in tiles)

    # pack per-core data: P[core][ch, p, col] = x_k[b, s, ch*128+p]
    packs = np.full((NCORES, CH, 128, R), NEG, dtype=np.float32)
    for k in range(KINDS):
        xk, eff, order = xs[k], effs[k], orders[k]
        for i in range(SLOTS):
            off = slot_cols[k][i]
            for c in range(NCORES):
                bidx = order[i * NCORES + c]
                e = int(eff[bidx])
                dst = packs[c].reshape(CH * 128, R)
                dst[:, off : off + e] = xk[bidx, :e, :].T

    # weights, laid out so lhsT slices are [128 (h), C] per (kind, chunk)
    wtp = np.ascontiguousarray(
        W.reshape(C, KINDS, CH, 128).transpose(3, 1, 2, 0)
    ).reshape(128, KINDS * CH, C)

    key = (R, tuple(groups_all), tuple((c0, c1) for c0, c1, _ in tiles))
    nc = _NC_CACHE.get(key)
    if nc is None:
        nc = _build_program(groups_all, R, tiles)
        _NC_CACHE[key] = nc
    in_maps = [{"p": packs[c], "wt": wtp} for c in range(NCORES)]
    res = run_bass_kernel_spmd(
        nc, in_maps, core_ids=list(range(NCORES)), **(_run_opts or {})
    )

    # combine per-row partials
    out_full = np.zeros((B, C), dtype=np.float32)
    res_all = np.stack([res.results[c]["out"] for c in range(NCORES)])  # [8, C, 128]
    for k in range(KINDS):
        part = res_all[:, :, k * SLOTS : (k + 1) * SLOTS]  # [core, C, slot]
        part = part.transpose(2, 0, 1).reshape(B, C)  # [(slot, core), C]
        out_full[orders[k]] += part
    out_full += b[None, :]
    if _run_opts is not None:
        kernel._last_res = res
    return out_full


# revision 12
# speedup vs baseline: 1.0988x; 1.0746x over previous
"""Trainium2 Bass kernel for nn_Classifier_42588895707508.

Computation (see reference):
    pool_k[b, h] = max_{s < eff_k[b]} x_k[b, s, h]      (k = 1, 2)
    out[b, c]    = sum_h pool_1[b,h] W[c,h] + pool_2[b,h] W[c, 768+h] + bias[c]
where eff_k[b] is derived from the mask m_k (index of first zero; 0 -> S).

Strategy (memory-bound, ragged sequences):
  * The valid region of each sample is a contiguous DRAM prefix. Only those
    bytes ever need to touch the device (~50% of the input on average).
  * Host computes eff from the masks, then packs, per core, the transposed
    valid blocks x_k[b, :eff, :].T (h on partitions, s on the free dim)
    into one dense array P[128, 6R]: each row's segment stores its 6
    h-chunks of 128 partitions back to back (chunk-major within the slot),
    all rows concatenated along the free dim.
  * Rows (= (kind, sample) pairs, 2*512 = 1024 of them) are distributed
    round-robin by length rank across the 8 cores, so every core gets an
    identical segment-width structure -> one SPMD program, perfectly
    balanced load.
  * Segments are grouped into equal-width groups (sorted by length, padded
    by at most a few columns) so one DVE reduce_max instruction with a
    [128, 6g, w] access pattern pools g rows x 6 chunks at once.
  * The tiny linear layer runs on the tensor engine (fp32 matmul,
    K=128 per chunk accumulated in PSUM); per-row partial dot products
    are summed on the host (x1/x2 rows of one sample may land on
    different cores).
"""

import numpy as np

B, S, H, C = 512, 256, 768, 2
NCORES = 8
CH = H // 128  # 6 h-chunks of 128 partitions
KINDS = 2
SLOTS = B // NCORES  # 64 slots per kind per core
NEG = np.float32(-3.4028235e38)

# grouping cost model (slot-column units): a padded slot-column costs ~1
# (6 real columns of DMA+DVE), an extra reduce instruction only eats DVE
# slack, so it is cheap
PAD_COST = 1.0
INSTR_COST = 4.0
TILE_W = 6144  # max free width (real columns) of one SBUF data tile
RAMP = [768, 1536, 3072]  # smaller first tiles so DVE starts early
DATA_BUFS = 5


def _eff_lengths(m):
    am = np.argmin(np.asarray(m), axis=1)
    return np.where(am == 0, S, am).astype(np.int64)


def _plan_groups(widths):
    """Partition the (descending) width list into contiguous groups.

    Returns list of (start, n, gw) minimizing PAD_COST * padding +
    INSTR_COST per group, via O(n^2) DP. A group must fit in one SBUF
    data tile: n * 6 * gw <= TILE_W.
    """
    n = len(widths)
    best = np.full(n + 1, np.inf)
    best[0] = 0.0
    prev = np.zeros(n + 1, dtype=np.int64)
    for i in range(1, n + 1):
        for j in range(i - 1, -1, -1):
            if (i - j) * 6 * widths[j] > TILE_W:
                break
            pad = PAD_COST * ((i - j) * widths[j] - widths[j:i].sum())
            cost = best[j] + pad + INSTR_COST
            if cost < best[i]:
                best[i] = cost
                prev[i] = j
    groups = []
    i = n
    while i > 0:
        j = prev[i]
        groups.append((j, i - j, int(widths[j])))
        i = j
    groups.reverse()
    return groups


def _build_program(groups_all, R, tiles):
    """Build the SPMD Bass program.

    groups_all: list of (col_off, out_slot, n, w); col_off in slot-column
    units (real DRAM column = 6 * slot-column). tiles: list of
    (c0, c1, [groups...]) in real columns, group boundaries inside.
    """
    import concourse.bacc as bacc
    import concourse.mybir as mybir
    from concourse.tile import TileContext

    nc = bacc.Bacc("TRN2", target_bir_lowering=False, debug=False, num_devices=NCORES)
    p_in = nc.dram_tensor("p", [128, 6 * R], mybir.dt.float32, kind="ExternalInput")
    wt_in = nc.dram_tensor(
        "wt", [128, KINDS * CH, C], mybir.dt.float32, kind="ExternalInput"
    )
    out_d = nc.dram_tensor(
        "out", [C, 2 * SLOTS], mybir.dt.float32, kind="ExternalOutput"
    )

    with TileContext(nc) as tc:
        with (
            tc.tile_pool(name="data", bufs=DATA_BUFS) as data_pool,
            tc.tile_pool(name="pooled", bufs=1) as pooled_pool,
            tc.tile_pool(name="small", bufs=1) as small_pool,
            tc.tile_pool(name="psum", bufs=1, space="PSUM") as psum_pool,
        ):
            wt_t = small_pool.tile([128, KINDS * CH, C], mybir.dt.float32, tag="wt")
            nc.sync.dma_start(out=wt_t, in_=wt_in[:, :, :])

            # pooled[p, slot, ch]: slot = kind*64 + i, partition p = h within chunk
            pooled = pooled_pool.tile(
                [128, KINDS * SLOTS, CH], mybir.dt.float32, tag="pooled", name="pooled"
            )

            for c0, c1, tgroups in tiles:
                tw = c1 - c0
                dt = data_pool.tile([128, TILE_W], mybir.dt.float32, tag="data")
                nc.sync.dma_start(out=dt[:, :tw], in_=p_in[:, c0:c1])
                for col_off, out_slot, gn, gw in tgroups:
                    a = 6 * col_off - c0
                    view = dt[:, a : a + gn * 6 * gw].rearrange(
                        "p (g w) -> p g w", w=gw
                    )
                    nc.vector.reduce_max(
                        out=pooled[:, out_slot : out_slot + gn, :],
                        in_=view,
                        axis=mybir.AxisListType.X,
                    )

            out_sb = small_pool.tile([C, 2 * SLOTS], mybir.dt.float32, tag="osb")
            for k in range(KINDS):
                ps = psum_pool.tile([C, SLOTS], mybir.dt.float32, tag=f"ps{k}")
                for ch in range(CH):
                    nc.tensor.matmul(
                        ps,
                        lhsT=wt_t[:, k * CH + ch, :],
                        rhs=pooled[:, k * SLOTS : (k + 1) * SLOTS, ch],
                        start=(ch == 0),
                        stop=(ch == CH - 1),
                    )
                nc.scalar.copy(
                    out=out_sb[:, k * SLOTS : (k + 1) * SLOTS], in_=ps
                )
            nc.sync.dma_start(out=out_d[:, :], in_=out_sb)

    nc.compile()
    return nc


_NC_CACHE = {}


def kernel(x1, x2, m1, m2, W, b, _run_opts=None):
    from concourse.bass_utils import run_bass_kernel_spmd

    x1 = np.asarray(x1)
    x2 = np.asarray(x2)
    W = np.asarray(W, dtype=np.float32)
    b = np.asarray(b, dtype=np.float32)
    xs = (x1, x2)
    effs = [_eff_lengths(m1), _eff_lengths(m2)]
    # descending length order per kind; slot i on core c holds sample
    # orders[k][i*NCORES + c]
    orders = [np.argsort(-effs[k], kind="stable") for k in range(KINDS)]
    # slot width = max eff among the 8 cores' samples of that slot
    slot_w = [
        effs[k][orders[k][:: NCORES]].astype(np.int64) for k in range(KINDS)
    ]  # [64] each, descending

    # group slots (per kind) into equal-width reduce groups
    raw_groups = []  # (kind, start, n, gw)
    for k in range(KINDS):
        for start, n, gw in _plan_groups(slot_w[k]):
            raw_groups.append((k, start, n, gw))
    # widest groups first; the short groups at the tail keep the final
    # reduce after the last DMA tile small
    raw_groups.sort(key=lambda g: -g[3])
    emit = raw_groups
    col = 0  # slot-column units
    groups_all = []  # (col_off, out_slot, n_slots, width) in emission order
    slot_cols = [np.zeros(SLOTS, dtype=np.int64) for _ in range(KINDS)]
    slot_gw = [np.zeros(SLOTS, dtype=np.int64) for _ in range(KINDS)]
    for k, start, n, gw in emit:
        groups_all.append((col, k * SLOTS + start, n, gw))
        for j in range(n):
            slot_cols[k][start + j] = col + j * gw
            slot_gw[k][start + j] = gw
        col += n * gw
    R = col

    # DMA tiles (real columns): whole groups per tile; the first few tiles
    # are smaller (RAMP) so the vector engine starts reducing early
    tiles = []
    cur = None
    for g in groups_all:
        col_off, _, gn, gw = g
        a0, a1 = 6 * col_off, 6 * (col_off + gn * gw)
        cap = RAMP[len(tiles)] if len(tiles) < len(RAMP) else TILE_W
        if cur is not None and (a1 - cur[0]) <= cap:
            cur[1] = a1
            cur[2].append(g)
        else:
            if cur is not None:
                tiles.append(tuple(cur))
            cur = [a0, a1, [g]]
    if cur is not None:
        tiles.append(tuple(cur))
    assert all(c1 - c0 <= TILE_W for c0, c1, _ in tiles)

    # pack per-core data: P[core][p, 6*off + ch*w + j] = x_k[b, j, ch*128+p]
    packs = np.full((NCORES, 128, 6 * R), NEG, dtype=np.float32)
    for k in range(KINDS):
        xk, eff, order = xs[k], effs[k], orders[k]
        for i in range(SLOTS):
            off = slot_cols[k][i]
            w = slot_gw[k][i]
            for c in range(NCORES):
                bidx = order[i * NCORES + c]
                e = int(eff[bidx])
                dst = packs[c][:, 6 * off : 6 * (off + w)].reshape(128, 6, w)
                src = xk[bidx, :e, :].T.reshape(6, 128, e)
                dst[:, :, :e] = src.transpose(1, 0, 2)

    # weights, laid out so lhsT slices are [128 (h), C] per (kind, chunk)
    wtp = np.ascontiguousarray(
        W.reshape(C, KINDS, CH, 128).transpose(3, 1, 2, 0)
    ).reshape(128, KINDS * CH, C)

    key = (R, tuple(groups_all), tuple((c0, c1) for c0, c1, _ in tiles))
    nc = _NC_CACHE.get(key)
    if nc is None:
        nc = _build_program(groups_all, R, tiles)
        _NC_CACHE[key] = nc
    in_maps = [{"p": packs[c], "wt": wtp} for c in range(NCORES)]

    res = None
    last_err = None
    for _attempt in range(3):
        try:
            res = run_bass_kernel_spmd(
                nc, in_maps, core_ids=list(range(NCORES)), **(_run_opts or {})
            )
            break
        except Exception as e:  # wedged device etc. -- retry
            last_err = e
    if res is None:
        raise last_err

    # combine per-row partials
    out_full = np.zeros((B, C), dtype=np.float32)
    res_all = np.stack([res.results[c]["out"] for c in range(NCORES)])  # [8, C, 128]
    for k in range(KINDS):
        part = res_all[:, :, k * SLOTS : (k + 1) * SLOTS]  # [core, C, slot]
        part = part.transpose(2, 0, 1).reshape(B, C)  # [(slot, core), C]
        out_full[orders[k]] += part
    out_full += b[None, :]
    if _run_opts is not None:
        kernel._last_res = res
    return out_full
